# revision 12
# baseline (speedup 1.0000x reference)
"""GQA attention (16 heads, 4 KV groups, S=2048, E=2048, HD=128) on 8 TRN2 cores.

Tensor-parallel by heads: core d owns heads {2d, 2d+1} and KV group d//2.
Each core computes its heads' attention + its slice of the output projection;
the host sums the 8 partial outputs (all-reduce) and concatenates the p slices.

Compute is bf16 on the TensorEngine (fp32 PSUM accumulation); RoPE and softmax
statistics are fp32. Outputs are stored bf16 and upconverted on host (verified
~0.5% rel err, well under the 2e-2 gate).

Outputs per core:
  p_out       [2, S, S] bf16 -- softmax probs for the core's 2 heads (causal;
                                the upper triangle is never written and relies
                                on the runtime pre-zeroing output buffers)
  out_partial [S, OUT]  bf16 -- this core's partial of o @ Wout
"""

import math
import os
from contextlib import ExitStack

import ml_dtypes
import numpy as np

import concourse.bass as bass
import concourse.bacc as bacc
import concourse.mybir as mybir
import concourse.tile as tile
from concourse.masks import make_causal_mask

F32 = mybir.dt.float32
BF16 = mybir.dt.bfloat16
AF = mybir.ActivationFunctionType

# Problem constants (hardcoded per the grading contract).
S = 2048          # sequence length
E = 2048          # embedding dim
HD = 128          # head dim
H = 16            # total heads
G = 4             # kv groups
N_CORES = 8
H_LOC = H // N_CORES       # 2 heads per core
OUT = H * HD               # 2048
P = 128                    # partitions
SCALE = 1.0 / math.sqrt(HD)
NEG = -1e9                 # additive causal mask value (pre-scale)


def build_nc(s=S, e=E):
    """Build the single-core Bass program (identical across cores; per-core
    behaviour comes entirely from the data each core receives)."""
    nt = s // P
    et = e // P
    qkvw = H_LOC * HD + 2 * HD          # fused QKV projection width: 512
    nq = H_LOC * HD                      # 256

    nc = bacc.Bacc(None)
    xT = nc.declare_dram_parameter("xT", [e, s], BF16, isOutput=False)
    w_qkv = nc.declare_dram_parameter("w_qkv", [e, qkvw], BF16, isOutput=False)
    w_out = nc.declare_dram_parameter("w_out", [nq, s], BF16, isOutput=False)
    cos_d = nc.declare_dram_parameter("cos", [s, HD], F32, isOutput=False)
    sin_d = nc.declare_dram_parameter("sin", [s, HD], F32, isOutput=False)
    p_out = nc.declare_dram_parameter("p_out", [H_LOC, s, s], BF16, isOutput=True)
    out_partial = nc.declare_dram_parameter("out_partial", [s, s], BF16, isOutput=True)

    xT_t = xT[:].rearrange("(a p) m -> p a m", p=P)          # [128, et, s]
    wqkv_t = w_qkv[:].rearrange("(a p) m -> p a m", p=P)     # [128, et, 512]
    wout_t = w_out[:].rearrange("(a p) m -> p a m", p=P)     # [128, 2, s]
    cos_t = cos_d[:].rearrange("(a p) m -> p a m", p=P)      # [128, nt, 128]
    sin_t = sin_d[:].rearrange("(a p) m -> p a m", p=P)

    with tile.TileContext(nc) as tc, ExitStack() as ctx:
        consts = ctx.enter_context(tc.tile_pool(name="consts", bufs=1))
        persist = ctx.enter_context(tc.tile_pool(name="persist", bufs=1))
        xin = ctx.enter_context(tc.tile_pool(name="xin", bufs=2))
        rope = ctx.enter_context(tc.tile_pool(name="rope", bufs=2))
        epool = ctx.enter_context(tc.tile_pool(name="epool", bufs=2))
        ppool = ctx.enter_context(tc.tile_pool(name="ppool", bufs=2))
        ptpool = ctx.enter_context(tc.tile_pool(name="ptpool", bufs=2))
        small = ctx.enter_context(tc.tile_pool(name="small", bufs=4))
        ostage = ctx.enter_context(tc.tile_pool(name="ostage", bufs=3))
        ps_mm = ctx.enter_context(tc.tile_pool(name="ps_mm", bufs=4, space="PSUM"))
        ps_o = ctx.enter_context(tc.tile_pool(name="ps_o", bufs=2, space="PSUM"))

        # ---- constants ----
        wqkv_sb = consts.tile([P, et, qkvw], BF16)
        nc.sync.dma_start(out=wqkv_sb, in_=wqkv_t)
        cos_sb = consts.tile([P, nt, HD], F32)
        nc.sync.dma_start(out=cos_sb, in_=cos_t)
        sin_sb = consts.tile([P, nt, HD], F32)
        nc.sync.dma_start(out=sin_sb, in_=sin_t)
        cmask = consts.tile([P, P], F32)
        make_causal_mask(nc, cmask, mask_val=NEG)

        # ---- persistent activations (bf16) ----
        qt_sb = persist.tile([P, H_LOC, s], BF16)   # Q'^T per head  [HD, s]
        kt_sb = persist.tile([P, s], BF16)          # K'^T           [HD, s]
        v_sb = persist.tile([P, nt, HD], BF16)      # V natural      [s, HD]
        ot_sb = persist.tile([P, H_LOC, s], BF16)   # o^T per head   [HD, s]

        # ================= Phase A: QKV projection + RoPE + transposes ======
        for i in range(nt):
            xcol = xin.tile([P, et, P], BF16)
            nc.sync.dma_start(out=xcol, in_=xT_t[:, :, i * P:(i + 1) * P])
            ps = ps_mm.tile([P, 512], F32, tag="ps_mm")
            for ei in range(et):
                nc.tensor.matmul(
                    ps[:, :qkvw], xcol[:, ei, :], wqkv_sb[:, ei, :],
                    start=(ei == 0), stop=(ei == et - 1),
                )
            # RoPE (fp32) on the Q (2 heads) + K segments; V is a plain copy.
            qk = rope.tile([P, nq + HD], F32)
            rot = rope.tile([P, nq + HD], F32)
            qkb = rope.tile([P, nq + HD], BF16)
            for j in range(H_LOC + 1):
                seg = ps[:, j * HD:(j + 1) * HD]
                c = cos_sb[:, i, :]
                sn = sin_sb[:, i, :]
                h2 = HD // 2
                # rot = [q_hi, q_lo] * sin_eff  (sin first half pre-negated on host)
                nc.vector.tensor_mul(rot[:, j * HD:j * HD + h2], seg[:, h2:], sn[:, :h2])
                nc.vector.tensor_mul(rot[:, j * HD + h2:(j + 1) * HD], seg[:, :h2], sn[:, h2:])
                nc.vector.tensor_mul(qk[:, j * HD:(j + 1) * HD], seg, c)
            nc.vector.tensor_add(qkb, qk, rot)
            nc.scalar.copy(v_sb[:, i, :], ps[:, nq + HD:])
            # Transpose rope'd Q/K blocks into [HD, seq] layouts via xbar DMA.
            for j in range(H_LOC):
                nc.sync.dma_start_transpose(
                    qt_sb[:, j, i * P:(i + 1) * P], qkb[:, j * HD:(j + 1) * HD]
                )
            nc.sync.dma_start_transpose(
                kt_sb[:, i * P:(i + 1) * P], qkb[:, H_LOC * HD:(H_LOC + 1) * HD]
            )

        # ================= Phase B: attention per head ======================
        for h in range(H_LOC):
            for qi in range(nt):
                klen = (qi + 1) * P
                nch = (klen + 511) // 512
                esb = epool.tile([P, s], BF16, tag="esb")
                acc = small.tile([P, 4], F32)
                for c in range(nch):
                    n0 = c * 512
                    ncw = min(512, klen - n0)
                    ps = ps_mm.tile([P, 512], F32, tag="ps_mm")
                    nc.tensor.matmul(
                        ps[:, :ncw],
                        qt_sb[:, h, qi * P:(qi + 1) * P],
                        kt_sb[:, n0:n0 + ncw],
                        start=True, stop=True,
                    )
                    if c == nch - 1:
                        nc.vector.tensor_add(ps[:, ncw - P:ncw], ps[:, ncw - P:ncw], cmask)
                    nc.scalar.activation(
                        esb[:, n0:n0 + ncw], ps[:, :ncw], AF.Exp,
                        scale=SCALE, accum_out=acc[:, c:c + 1],
                    )
                den = small.tile([P, 1], F32)
                nc.vector.reduce_sum(den, acc[:, :nch], axis=mybir.AxisListType.X)
                rden = small.tile([P, 1], F32)
                nc.vector.reciprocal(rden, den)
                pbf = ppool.tile([P, s], BF16, tag="pbf")
                nc.gpsimd.tensor_scalar_mul(pbf[:, :klen], esb[:, :klen], rden)
                nc.sync.dma_start(
                    out=p_out[h, qi * P:(qi + 1) * P, 0:klen], in_=pbf[:, :klen]
                )
                # Transpose P into the [k, q] staging buffer via xbar DMA.
                if qi % 2 == 0:
                    pt = ptpool.tile([P, nt, 2 * P], BF16, tag="pt")
                    nc.gpsimd.memset(pt[:, qi + 1, 0:P], 0.0)
                half = (qi % 2) * P
                nc.sync.dma_start_transpose(
                    pt[:, 0:qi + 1, half:half + P], pbf[:, :klen]
                )
                if qi % 2 == 1:
                    m = qi // 2
                    pso = ps_o.tile([P, 2 * P], F32)
                    for kt in range(qi + 1):
                        nc.tensor.matmul(
                            pso, v_sb[:, kt, :], pt[:, kt, :],
                            start=(kt == 0), stop=(kt == qi),
                        )
                    nc.scalar.copy(ot_sb[:, h, m * 2 * P:(m + 1) * 2 * P], pso)

        # ================= Phase C: output projection =======================
        wout_sb = consts.tile([P, nq // P, s], BF16)
        nc.sync.dma_start(out=wout_sb, in_=wout_t)
        for qi in range(nt):
            for c in range(s // 512):
                ps = ps_mm.tile([P, 512], F32, tag="ps_mm")
                for t in range(nq // P):
                    nc.tensor.matmul(
                        ps, ot_sb[:, t, qi * P:(qi + 1) * P],
                        wout_sb[:, t, c * 512:(c + 1) * 512],
                        start=(t == 0), stop=(t == nq // P - 1),
                    )
                ost = ostage.tile([P, 512], BF16)
                if (qi * (s // 512) + c) % 2 == 0:
                    nc.scalar.copy(ost, ps)
                else:
                    nc.vector.tensor_copy(ost, ps)
                nc.sync.dma_start(
                    out=out_partial[qi * P:(qi + 1) * P, c * 512:(c + 1) * 512],
                    in_=ost,
                )

    return nc


_NC_CACHE = {}


def _get_nc():
    key = "full"
    if key not in _NC_CACHE:
        nc = build_nc()
        nc.finalize()
        _NC_CACHE[key] = nc
    return _NC_CACHE[key]


def shard_inputs(x, cos, sin, Wq, Wk, Wv, Wout):
    """Host-side sharding: per-core input dicts (tensor-parallel by heads)."""
    bf = ml_dtypes.bfloat16
    xT = np.ascontiguousarray(np.asarray(x, dtype=np.float32)[0].T.astype(bf))
    cos = np.ascontiguousarray(np.asarray(cos, dtype=np.float32))
    sin = np.asarray(sin, dtype=np.float32)
    h2 = HD // 2
    sin_eff = np.ascontiguousarray(
        np.concatenate([-sin[:, :h2], sin[:, h2:]], axis=1)
    )
    Wq = np.asarray(Wq, dtype=np.float32)
    Wk = np.asarray(Wk, dtype=np.float32)
    Wv = np.asarray(Wv, dtype=np.float32)
    Wout = np.asarray(Wout, dtype=np.float32)
    in_maps = []
    for d in range(N_CORES):
        h0 = d * H_LOC
        g = h0 * G // H
        w_qkv = np.ascontiguousarray(np.concatenate(
            [
                Wq[:, h0 * HD:(h0 + H_LOC) * HD],
                Wk[:, g * HD:(g + 1) * HD],
                Wv[:, g * HD:(g + 1) * HD],
            ],
            axis=1,
        ).astype(bf))
        w_out = np.ascontiguousarray(Wout[h0 * HD:(h0 + H_LOC) * HD, :].astype(bf))
        in_maps.append({
            "xT": xT, "w_qkv": w_qkv, "w_out": w_out,
            "cos": cos, "sin": sin_eff,
        })
    return in_maps


def _ensure_ntff_hook():
    """Register the axon NTFF profile hook if the image's antenv lacks it."""
    import sys
    import types

    try:
        from antenv.axon_hooks import get_axon_ntff_profile_hook  # noqa: F401
        return
    except ImportError:
        pass
    mod = types.ModuleType("antenv.axon_hooks")
    mod._hook = None

    def set_axon_ntff_profile_hook(h):
        mod._hook = h

    def get_axon_ntff_profile_hook():
        return mod._hook

    mod.set_axon_ntff_profile_hook = set_axon_ntff_profile_hook
    mod.get_axon_ntff_profile_hook = get_axon_ntff_profile_hook
    sys.modules["antenv.axon_hooks"] = mod
    import antenv

    antenv.axon_hooks = mod
    so = "/opt/axon/libaxon_pjrt.so"
    if os.path.exists(so):
        try:
            from trn_agent_boot.trn_boot import _ntff_profile_via_ctypes

            set_axon_ntff_profile_hook(_ntff_profile_via_ctypes(so))
        except Exception as exc:  # pragma: no cover
            print(f"ntff hook registration failed: {exc}")
    import concourse.bass_utils as bu

    bu.upload_artifacts = lambda tmpdir: tmpdir


def kernel(x, mask, cos, sin, Wq, Wk, Wv, Wout):
    from concourse.bass_utils import run_bass_kernel_spmd

    nc = _get_nc()
    in_maps = shard_inputs(x, cos, sin, Wq, Wk, Wv, Wout)
    trace = bool(os.environ.get("BASS_KERNEL_TRACE"))
    if trace:
        _ensure_ntff_hook()
    res = run_bass_kernel_spmd(
        nc, in_maps, core_ids=list(range(N_CORES)), trace=trace,
        tmpdir=os.environ.get("BASS_KERNEL_TMPDIR"),
    )
    if trace and res.exec_time_ns is not None:
        print(f"HW exec time: {res.exec_time_ns} ns")
    out = np.zeros((S, OUT), dtype=np.float32)
    for d in range(N_CORES):
        out += res.results[d]["out_partial"].astype(np.float32)
    p = np.concatenate(
        [res.results[d]["p_out"].astype(np.float32) for d in range(N_CORES)], axis=0
    )
    return out[None], p[None]


# revision 13
# speedup vs baseline: 1.7254x; 1.7254x over previous
"""GQA attention (16 heads, 4 KV groups, S=2048, E=2048, HD=128) on 8 TRN2 cores.

Tensor-parallel by heads: core d owns heads {2d, 2d+1} and KV group d//2.
Each core computes its heads' attention + its slice of the output projection;
the host sums the 8 partial outputs (all-reduce) and concatenates the p slices.

Compute is bf16 on the TensorEngine (fp32 PSUM accumulation); RoPE and softmax
statistics are fp32. Outputs are stored bf16 and upconverted on host (verified
~0.5% rel err, well under the 2e-2 gate).

Outputs per core:
  p_out       [2, S, S] bf16 -- softmax probs for the core's 2 heads (causal;
                                the upper triangle is never written and relies
                                on the runtime pre-zeroing output buffers)
  out_partial [S, OUT]  bf16 -- this core's partial of o @ Wout
"""

import math
import os
from contextlib import ExitStack

import ml_dtypes
import numpy as np

import concourse.bass as bass
import concourse.bacc as bacc
import concourse.mybir as mybir
import concourse.tile as tile
from concourse.masks import make_causal_mask

F32 = mybir.dt.float32
BF16 = mybir.dt.bfloat16
AF = mybir.ActivationFunctionType

# Problem constants (hardcoded per the grading contract).
S = 2048          # sequence length
E = 2048          # embedding dim
HD = 128          # head dim
H = 16            # total heads
G = 4             # kv groups
N_CORES = 8
H_LOC = H // N_CORES       # 2 heads per core
OUT = H * HD               # 2048
P = 128                    # partitions
SCALE = 1.0 / math.sqrt(HD)
NEG = -1e9                 # additive causal mask value (pre-scale)


def build_nc(s=S, e=E):
    """Build the single-core Bass program (identical across cores; per-core
    behaviour comes entirely from the data each core receives)."""
    nt = s // P
    et = e // P
    qkvw = H_LOC * HD + 2 * HD          # fused QKV projection width: 512
    nq = H_LOC * HD                      # 256

    nc = bacc.Bacc(None)
    xT = nc.declare_dram_parameter("xT", [e, s], BF16, isOutput=False)
    w_qkv = nc.declare_dram_parameter("w_qkv", [e, qkvw], BF16, isOutput=False)
    w_out = nc.declare_dram_parameter("w_out", [nq, s], BF16, isOutput=False)
    cos_d = nc.declare_dram_parameter("cos", [s, HD], F32, isOutput=False)
    sin_d = nc.declare_dram_parameter("sin", [s, HD], F32, isOutput=False)
    p_out = nc.declare_dram_parameter("p_out", [H_LOC, s, s], BF16, isOutput=True)
    out_partial = nc.declare_dram_parameter("out_partial", [s, s], BF16, isOutput=True)

    xT_t = xT[:].rearrange("(a p) m -> p a m", p=P)          # [128, et, s]
    wqkv_t = w_qkv[:].rearrange("(a p) m -> p a m", p=P)     # [128, et, 512]
    wout_t = w_out[:].rearrange("(a p) m -> p a m", p=P)     # [128, 2, s]
    cos_t = cos_d[:].rearrange("(a p) m -> p a m", p=P)      # [128, nt, 128]
    sin_t = sin_d[:].rearrange("(a p) m -> p a m", p=P)

    with tile.TileContext(nc) as tc, ExitStack() as ctx:
        consts = ctx.enter_context(tc.tile_pool(name="consts", bufs=1))
        persist = ctx.enter_context(tc.tile_pool(name="persist", bufs=1))
        xin = ctx.enter_context(tc.tile_pool(name="xin", bufs=2))
        rope = ctx.enter_context(tc.tile_pool(name="rope", bufs=2))
        epool = ctx.enter_context(tc.tile_pool(name="epool", bufs=2))
        ppool = ctx.enter_context(tc.tile_pool(name="ppool", bufs=2))
        ptpool = ctx.enter_context(tc.tile_pool(name="ptpool", bufs=2))
        small = ctx.enter_context(tc.tile_pool(name="small", bufs=4))
        ostage = ctx.enter_context(tc.tile_pool(name="ostage", bufs=3))
        ps_mm = ctx.enter_context(tc.tile_pool(name="ps_mm", bufs=4, space="PSUM"))
        ps_o = ctx.enter_context(tc.tile_pool(name="ps_o", bufs=2, space="PSUM"))

        # ---- constants ----
        wqkv_sb = consts.tile([P, et, qkvw], BF16)
        nc.sync.dma_start(out=wqkv_sb, in_=wqkv_t)
        cos_sb = consts.tile([P, nt, HD], F32)
        nc.sync.dma_start(out=cos_sb, in_=cos_t)
        sin_sb = consts.tile([P, nt, HD], F32)
        nc.sync.dma_start(out=sin_sb, in_=sin_t)
        cmask = consts.tile([P, P], F32)
        make_causal_mask(nc, cmask, mask_val=NEG)

        # ---- persistent activations (bf16) ----
        qt_sb = persist.tile([P, H_LOC, s], BF16)   # Q'^T per head  [HD, s]
        kt_sb = persist.tile([P, s], BF16)          # K'^T           [HD, s]
        v_sb = persist.tile([P, nt, HD], BF16)      # V natural      [s, HD]
        ot_sb = persist.tile([P, H_LOC, s], BF16)   # o^T per head   [HD, s]

        # ================= Phase A: QKV projection + RoPE + transposes ======
        for i in range(nt):
            xcol = xin.tile([P, et, P], BF16)
            nc.sync.dma_start(out=xcol, in_=xT_t[:, :, i * P:(i + 1) * P])
            ps = ps_mm.tile([P, 512], F32, tag="ps_mm")
            for ei in range(et):
                nc.tensor.matmul(
                    ps[:, :qkvw], xcol[:, ei, :], wqkv_sb[:, ei, :],
                    start=(ei == 0), stop=(ei == et - 1),
                )
            # RoPE (fp32) on the Q (2 heads) + K segments; V is a plain copy.
            qk = rope.tile([P, nq + HD], F32)
            rot = rope.tile([P, nq + HD], F32)
            qkb = rope.tile([P, nq + HD], BF16)
            for j in range(H_LOC + 1):
                seg = ps[:, j * HD:(j + 1) * HD]
                c = cos_sb[:, i, :]
                sn = sin_sb[:, i, :]
                h2 = HD // 2
                # rot = [q_hi, q_lo] * sin_eff  (sin first half pre-negated on host)
                nc.vector.tensor_mul(rot[:, j * HD:j * HD + h2], seg[:, h2:], sn[:, :h2])
                nc.vector.tensor_mul(rot[:, j * HD + h2:(j + 1) * HD], seg[:, :h2], sn[:, h2:])
                nc.vector.tensor_mul(qk[:, j * HD:(j + 1) * HD], seg, c)
            nc.vector.tensor_add(qkb, qk, rot)
            nc.scalar.copy(v_sb[:, i, :], ps[:, nq + HD:])
            # Transpose rope'd Q/K blocks into [HD, seq] layouts via xbar DMA.
            for j in range(H_LOC):
                nc.sync.dma_start_transpose(
                    qt_sb[:, j, i * P:(i + 1) * P], qkb[:, j * HD:(j + 1) * HD]
                )
            nc.sync.dma_start_transpose(
                kt_sb[:, i * P:(i + 1) * P], qkb[:, H_LOC * HD:(H_LOC + 1) * HD]
            )

        # ================= Phase B: attention per head ======================
        for h in range(H_LOC):
            for qi in range(nt):
                klen = (qi + 1) * P
                nch = (klen + 511) // 512
                esb = epool.tile([P, s], BF16, tag="esb")
                acc = small.tile([P, 4], F32)
                for c in range(nch):
                    n0 = c * 512
                    ncw = min(512, klen - n0)
                    ps = ps_mm.tile([P, 512], F32, tag="ps_mm")
                    nc.tensor.matmul(
                        ps[:, :ncw],
                        qt_sb[:, h, qi * P:(qi + 1) * P],
                        kt_sb[:, n0:n0 + ncw],
                        start=True, stop=True,
                    )
                    if c == nch - 1:
                        nc.vector.tensor_add(ps[:, ncw - P:ncw], ps[:, ncw - P:ncw], cmask)
                    nc.scalar.activation(
                        esb[:, n0:n0 + ncw], ps[:, :ncw], AF.Exp,
                        scale=SCALE, accum_out=acc[:, c:c + 1],
                    )
                den = small.tile([P, 1], F32)
                nc.vector.reduce_sum(den, acc[:, :nch], axis=mybir.AxisListType.X)
                rden = small.tile([P, 1], F32)
                nc.vector.reciprocal(rden, den)
                pbf = ppool.tile([P, s], BF16, tag="pbf")
                nc.vector.tensor_scalar_mul(pbf[:, :klen], esb[:, :klen], rden)
                nc.sync.dma_start(
                    out=p_out[h, qi * P:(qi + 1) * P, 0:klen], in_=pbf[:, :klen]
                )
                # Transpose P into the [k, q] staging buffer via xbar DMA.
                if qi % 2 == 0:
                    pt = ptpool.tile([P, nt, 2 * P], BF16, tag="pt")
                    nc.gpsimd.memset(pt[:, qi + 1, 0:P], 0.0)
                half = (qi % 2) * P
                nc.sync.dma_start_transpose(
                    pt[:, 0:qi + 1, half:half + P], pbf[:, :klen]
                )
                if qi % 2 == 1:
                    m = qi // 2
                    pso = ps_o.tile([P, 2 * P], F32)
                    for kt in range(qi + 1):
                        nc.tensor.matmul(
                            pso, v_sb[:, kt, :], pt[:, kt, :],
                            start=(kt == 0), stop=(kt == qi),
                        )
                    nc.scalar.copy(ot_sb[:, h, m * 2 * P:(m + 1) * 2 * P], pso)

        # ================= Phase C: output projection =======================
        wout_sb = consts.tile([P, nq // P, s], BF16)
        nc.sync.dma_start(out=wout_sb, in_=wout_t)
        for qi in range(nt):
            for c in range(s // 512):
                ps = ps_mm.tile([P, 512], F32, tag="ps_mm")
                for t in range(nq // P):
                    nc.tensor.matmul(
                        ps, ot_sb[:, t, qi * P:(qi + 1) * P],
                        wout_sb[:, t, c * 512:(c + 1) * 512],
                        start=(t == 0), stop=(t == nq // P - 1),
                    )
                ost = ostage.tile([P, 512], BF16)
                if (qi * (s // 512) + c) % 2 == 0:
                    nc.scalar.copy(ost, ps)
                else:
                    nc.vector.tensor_copy(ost, ps)
                nc.sync.dma_start(
                    out=out_partial[qi * P:(qi + 1) * P, c * 512:(c + 1) * 512],
                    in_=ost,
                )

    return nc


_NC_CACHE = {}


def _get_nc():
    key = "full"
    if key not in _NC_CACHE:
        nc = build_nc()
        nc.finalize()
        _NC_CACHE[key] = nc
    return _NC_CACHE[key]


def shard_inputs(x, cos, sin, Wq, Wk, Wv, Wout):
    """Host-side sharding: per-core input dicts (tensor-parallel by heads)."""
    bf = ml_dtypes.bfloat16
    xT = np.ascontiguousarray(np.asarray(x, dtype=np.float32)[0].T.astype(bf))
    cos = np.ascontiguousarray(np.asarray(cos, dtype=np.float32))
    sin = np.asarray(sin, dtype=np.float32)
    h2 = HD // 2
    sin_eff = np.ascontiguousarray(
        np.concatenate([-sin[:, :h2], sin[:, h2:]], axis=1)
    )
    Wq = np.asarray(Wq, dtype=np.float32)
    Wk = np.asarray(Wk, dtype=np.float32)
    Wv = np.asarray(Wv, dtype=np.float32)
    Wout = np.asarray(Wout, dtype=np.float32)
    in_maps = []
    for d in range(N_CORES):
        h0 = d * H_LOC
        g = h0 * G // H
        w_qkv = np.ascontiguousarray(np.concatenate(
            [
                Wq[:, h0 * HD:(h0 + H_LOC) * HD],
                Wk[:, g * HD:(g + 1) * HD],
                Wv[:, g * HD:(g + 1) * HD],
            ],
            axis=1,
        ).astype(bf))
        w_out = np.ascontiguousarray(Wout[h0 * HD:(h0 + H_LOC) * HD, :].astype(bf))
        in_maps.append({
            "xT": xT, "w_qkv": w_qkv, "w_out": w_out,
            "cos": cos, "sin": sin_eff,
        })
    return in_maps


def _ensure_ntff_hook():
    """Register the axon NTFF profile hook if the image's antenv lacks it."""
    import sys
    import types

    try:
        from antenv.axon_hooks import get_axon_ntff_profile_hook  # noqa: F401
        return
    except ImportError:
        pass
    mod = types.ModuleType("antenv.axon_hooks")
    mod._hook = None

    def set_axon_ntff_profile_hook(h):
        mod._hook = h

    def get_axon_ntff_profile_hook():
        return mod._hook

    mod.set_axon_ntff_profile_hook = set_axon_ntff_profile_hook
    mod.get_axon_ntff_profile_hook = get_axon_ntff_profile_hook
    sys.modules["antenv.axon_hooks"] = mod
    import antenv

    antenv.axon_hooks = mod
    so = "/opt/axon/libaxon_pjrt.so"
    if os.path.exists(so):
        try:
            from trn_agent_boot.trn_boot import _ntff_profile_via_ctypes

            set_axon_ntff_profile_hook(_ntff_profile_via_ctypes(so))
        except Exception as exc:  # pragma: no cover
            print(f"ntff hook registration failed: {exc}")
    import concourse.bass_utils as bu

    bu.upload_artifacts = lambda tmpdir: tmpdir


def kernel(x, mask, cos, sin, Wq, Wk, Wv, Wout):
    from concourse.bass_utils import run_bass_kernel_spmd

    nc = _get_nc()
    in_maps = shard_inputs(x, cos, sin, Wq, Wk, Wv, Wout)
    trace = bool(os.environ.get("BASS_KERNEL_TRACE"))
    if trace:
        _ensure_ntff_hook()
    res = run_bass_kernel_spmd(
        nc, in_maps, core_ids=list(range(N_CORES)), trace=trace,
        tmpdir=os.environ.get("BASS_KERNEL_TMPDIR"),
    )
    if trace and res.exec_time_ns is not None:
        print(f"HW exec time: {res.exec_time_ns} ns")
    out = np.zeros((S, OUT), dtype=np.float32)
    for d in range(N_CORES):
        out += res.results[d]["out_partial"].astype(np.float32)
    p = np.concatenate(
        [res.results[d]["p_out"].astype(np.float32) for d in range(N_CORES)], axis=0
    )
    return out[None], p[None]


# revision 15
# speedup vs baseline: 2.0728x; 1.2013x over previous
"""GQA attention (16 heads, 4 KV groups, S=2048, E=2048, HD=128) on 8 TRN2 cores.

Tensor-parallel by heads: core d owns heads {2d, 2d+1} and KV group d//2.
Each core computes its heads' attention + its slice of the output projection;
the host sums the 8 partial outputs (all-reduce) and concatenates the p slices.

Compute is bf16 on the TensorEngine (fp32 PSUM accumulation); RoPE and softmax
statistics are fp32. Outputs are stored bf16 and upconverted on host (verified
~0.5% rel err, well under the 2e-2 gate).

Outputs per core:
  p_out       [2, S, S] bf16 -- softmax probs for the core's 2 heads (causal;
                                the upper triangle is never written and relies
                                on the runtime pre-zeroing output buffers)
  out_partial [S, OUT]  bf16 -- this core's partial of o @ Wout
"""

import math
import os
from contextlib import ExitStack

import ml_dtypes
import numpy as np

import concourse.bass as bass
import concourse.bacc as bacc
import concourse.mybir as mybir
import concourse.tile as tile
from concourse.masks import make_causal_mask, make_identity

F32 = mybir.dt.float32
BF16 = mybir.dt.bfloat16
AF = mybir.ActivationFunctionType

# Problem constants (hardcoded per the grading contract).
S = 2048          # sequence length
E = 2048          # embedding dim
HD = 128          # head dim
H = 16            # total heads
G = 4             # kv groups
N_CORES = 8
H_LOC = H // N_CORES       # 2 heads per core
OUT = H * HD               # 2048
P = 128                    # partitions
SCALE = 1.0 / math.sqrt(HD)
NEG = -1e9                 # additive causal mask value (pre-scale)


def build_nc(s=S, e=E):
    """Build the single-core Bass program (identical across cores; per-core
    behaviour comes entirely from the data each core receives)."""
    nt = s // P
    et = e // P
    qkvw = H_LOC * HD + 2 * HD          # fused QKV projection width: 512
    nq = H_LOC * HD                      # 256

    nc = bacc.Bacc(None)
    xT = nc.declare_dram_parameter("xT", [e, s], BF16, isOutput=False)
    w_qkv = nc.declare_dram_parameter("w_qkv", [e, qkvw], BF16, isOutput=False)
    w_out = nc.declare_dram_parameter("w_out", [nq, s], BF16, isOutput=False)
    cos_d = nc.declare_dram_parameter("cos", [s, HD], F32, isOutput=False)
    sin_d = nc.declare_dram_parameter("sin", [s, HD], F32, isOutput=False)
    p_out = nc.declare_dram_parameter("p_out", [H_LOC, s, s], BF16, isOutput=True)
    out_partial = nc.declare_dram_parameter("out_partial", [s, s], BF16, isOutput=True)

    xT_t = xT[:].rearrange("(a p) m -> p a m", p=P)          # [128, et, s]
    wqkv_t = w_qkv[:].rearrange("(a p) m -> p a m", p=P)     # [128, et, 512]
    wout_t = w_out[:].rearrange("(a p) m -> p a m", p=P)     # [128, 2, s]
    cos_t = cos_d[:].rearrange("(a p) m -> p a m", p=P)      # [128, nt, 128]
    sin_t = sin_d[:].rearrange("(a p) m -> p a m", p=P)

    with tile.TileContext(nc) as tc, ExitStack() as ctx:
        consts = ctx.enter_context(tc.tile_pool(name="consts", bufs=1))
        persist = ctx.enter_context(tc.tile_pool(name="persist", bufs=1))
        xin = ctx.enter_context(tc.tile_pool(name="xin", bufs=2))
        rope = ctx.enter_context(tc.tile_pool(name="rope", bufs=2))
        epool = ctx.enter_context(tc.tile_pool(name="epool", bufs=2))
        ppool = ctx.enter_context(tc.tile_pool(name="ppool", bufs=2))
        ptpool = ctx.enter_context(tc.tile_pool(name="ptpool", bufs=2))
        small = ctx.enter_context(tc.tile_pool(name="small", bufs=4))
        ostage = ctx.enter_context(tc.tile_pool(name="ostage", bufs=3))
        ps_mm = ctx.enter_context(tc.tile_pool(name="ps_mm", bufs=4, space="PSUM"))
        ps_o = ctx.enter_context(tc.tile_pool(name="ps_o", bufs=2, space="PSUM"))
        ps_t = ctx.enter_context(tc.tile_pool(name="ps_t", bufs=2, space="PSUM"))

        # ---- constants ----
        wqkv_sb = consts.tile([P, et, qkvw], BF16)
        nc.sync.dma_start(out=wqkv_sb, in_=wqkv_t)
        cos_sb = consts.tile([P, nt, HD], F32)
        nc.sync.dma_start(out=cos_sb, in_=cos_t)
        sin_sb = consts.tile([P, nt, HD], F32)
        nc.sync.dma_start(out=sin_sb, in_=sin_t)
        ident = consts.tile([P, P], BF16)
        make_identity(nc, ident)
        cmask = consts.tile([P, P], F32)
        make_causal_mask(nc, cmask, mask_val=NEG)

        # ---- persistent activations (bf16) ----
        qt_sb = persist.tile([P, H_LOC, s], BF16)   # Q'^T per head  [HD, s]
        kt_sb = persist.tile([P, s], BF16)          # K'^T           [HD, s]
        v_sb = persist.tile([P, nt, HD], BF16)      # V natural      [s, HD]
        ot_sb = persist.tile([P, H_LOC, s], BF16)   # o^T per head   [HD, s]

        # ================= Phase A: QKV projection + RoPE + transposes ======
        for i in range(nt):
            xcol = xin.tile([P, et, P], BF16)
            nc.sync.dma_start(out=xcol, in_=xT_t[:, :, i * P:(i + 1) * P])
            ps = ps_mm.tile([P, 512], F32, tag="ps_mm")
            for ei in range(et):
                nc.tensor.matmul(
                    ps[:, :qkvw], xcol[:, ei, :], wqkv_sb[:, ei, :],
                    start=(ei == 0), stop=(ei == et - 1),
                )
            # RoPE (fp32) on the Q (2 heads) + K segments; V is a plain copy.
            qk = rope.tile([P, nq + HD], F32)
            rot = rope.tile([P, nq + HD], F32)
            qkb = rope.tile([P, nq + HD], BF16)
            for j in range(H_LOC + 1):
                seg = ps[:, j * HD:(j + 1) * HD]
                c = cos_sb[:, i, :]
                sn = sin_sb[:, i, :]
                h2 = HD // 2
                # rot = [q_hi, q_lo] * sin_eff  (sin first half pre-negated on host)
                nc.vector.tensor_mul(rot[:, j * HD:j * HD + h2], seg[:, h2:], sn[:, :h2])
                nc.vector.tensor_mul(rot[:, j * HD + h2:(j + 1) * HD], seg[:, :h2], sn[:, h2:])
                nc.vector.tensor_mul(qk[:, j * HD:(j + 1) * HD], seg, c)
            nc.vector.tensor_add(qkb, qk, rot)
            nc.scalar.copy(v_sb[:, i, :], ps[:, nq + HD:])
            # Transpose rope'd Q/K blocks into [HD, seq] layouts (PE + DVE).
            pst = ps_t.tile([P, (H_LOC + 1) * P], BF16)
            for j in range(H_LOC + 1):
                nc.tensor.transpose(
                    pst[:, j * P:(j + 1) * P], qkb[:, j * HD:(j + 1) * HD], ident
                )
            nc.vector.tensor_copy(
                qt_sb[:, :, i * P:(i + 1) * P],
                pst[:, 0:H_LOC * P].rearrange("p (a m) -> p a m", m=P),
            )
            nc.vector.tensor_copy(
                kt_sb[:, i * P:(i + 1) * P], pst[:, H_LOC * P:(H_LOC + 1) * P]
            )

        # ================= Phase B: attention per head ======================
        for h in range(H_LOC):
            for qi in range(nt):
                klen = (qi + 1) * P
                nch = (klen + 511) // 512
                esb = epool.tile([P, s], BF16, tag="esb")
                acc = small.tile([P, 4], F32)
                for c in range(nch):
                    n0 = c * 512
                    ncw = min(512, klen - n0)
                    ps = ps_mm.tile([P, 512], F32, tag="ps_mm")
                    nc.tensor.matmul(
                        ps[:, :ncw],
                        qt_sb[:, h, qi * P:(qi + 1) * P],
                        kt_sb[:, n0:n0 + ncw],
                        start=True, stop=True,
                    )
                    if c == nch - 1:
                        nc.vector.tensor_add(ps[:, ncw - P:ncw], ps[:, ncw - P:ncw], cmask)
                    nc.scalar.activation(
                        esb[:, n0:n0 + ncw], ps[:, :ncw], AF.Exp,
                        scale=SCALE, accum_out=acc[:, c:c + 1],
                    )
                den = small.tile([P, 1], F32)
                nc.vector.reduce_sum(den, acc[:, :nch], axis=mybir.AxisListType.X)
                rden = small.tile([P, 1], F32)
                nc.vector.reciprocal(rden, den)
                pbf = ppool.tile([P, s], BF16, tag="pbf")
                nc.vector.tensor_scalar_mul(pbf[:, :klen], esb[:, :klen], rden)
                nc.sync.dma_start(
                    out=p_out[h, qi * P:(qi + 1) * P, 0:klen], in_=pbf[:, :klen]
                )
                # Transpose P into the [k, q] staging buffer via xbar DMA.
                if qi % 2 == 0:
                    pt = ptpool.tile([P, nt, 2 * P], BF16, tag="pt")
                    nc.gpsimd.memset(pt[:, qi + 1, 0:P], 0.0)
                half = (qi % 2) * P
                nc.scalar.dma_start_transpose(
                    pt[:, 0:qi + 1, half:half + P], pbf[:, :klen]
                )
                if qi % 2 == 1:
                    m = qi // 2
                    pso = ps_o.tile([P, 2 * P], F32)
                    for kt in range(qi + 1):
                        nc.tensor.matmul(
                            pso, v_sb[:, kt, :], pt[:, kt, :],
                            start=(kt == 0), stop=(kt == qi),
                        )
                    nc.scalar.copy(ot_sb[:, h, m * 2 * P:(m + 1) * 2 * P], pso)

        # ================= Phase C: output projection =======================
        wout_sb = consts.tile([P, nq // P, s], BF16)
        nc.sync.dma_start(out=wout_sb, in_=wout_t)
        for qi in range(nt):
            for c in range(s // 512):
                ps = ps_mm.tile([P, 512], F32, tag="ps_mm")
                for t in range(nq // P):
                    nc.tensor.matmul(
                        ps, ot_sb[:, t, qi * P:(qi + 1) * P],
                        wout_sb[:, t, c * 512:(c + 1) * 512],
                        start=(t == 0), stop=(t == nq // P - 1),
                    )
                ost = ostage.tile([P, 512], BF16)
                if (qi * (s // 512) + c) % 2 == 0:
                    nc.scalar.copy(ost, ps)
                else:
                    nc.vector.tensor_copy(ost, ps)
                nc.sync.dma_start(
                    out=out_partial[qi * P:(qi + 1) * P, c * 512:(c + 1) * 512],
                    in_=ost,
                )

    return nc


_NC_CACHE = {}


def _get_nc():
    key = "full"
    if key not in _NC_CACHE:
        nc = build_nc()
        nc.finalize()
        _NC_CACHE[key] = nc
    return _NC_CACHE[key]


def shard_inputs(x, cos, sin, Wq, Wk, Wv, Wout):
    """Host-side sharding: per-core input dicts (tensor-parallel by heads)."""
    bf = ml_dtypes.bfloat16
    xT = np.ascontiguousarray(np.asarray(x, dtype=np.float32)[0].T.astype(bf))
    cos = np.ascontiguousarray(np.asarray(cos, dtype=np.float32))
    sin = np.asarray(sin, dtype=np.float32)
    h2 = HD // 2
    sin_eff = np.ascontiguousarray(
        np.concatenate([-sin[:, :h2], sin[:, h2:]], axis=1)
    )
    Wq = np.asarray(Wq, dtype=np.float32)
    Wk = np.asarray(Wk, dtype=np.float32)
    Wv = np.asarray(Wv, dtype=np.float32)
    Wout = np.asarray(Wout, dtype=np.float32)
    in_maps = []
    for d in range(N_CORES):
        h0 = d * H_LOC
        g = h0 * G // H
        w_qkv = np.ascontiguousarray(np.concatenate(
            [
                Wq[:, h0 * HD:(h0 + H_LOC) * HD],
                Wk[:, g * HD:(g + 1) * HD],
                Wv[:, g * HD:(g + 1) * HD],
            ],
            axis=1,
        ).astype(bf))
        w_out = np.ascontiguousarray(Wout[h0 * HD:(h0 + H_LOC) * HD, :].astype(bf))
        in_maps.append({
            "xT": xT, "w_qkv": w_qkv, "w_out": w_out,
            "cos": cos, "sin": sin_eff,
        })
    return in_maps


def _ensure_ntff_hook():
    """Register the axon NTFF profile hook if the image's antenv lacks it."""
    import sys
    import types

    try:
        from antenv.axon_hooks import get_axon_ntff_profile_hook  # noqa: F401
        return
    except ImportError:
        pass
    mod = types.ModuleType("antenv.axon_hooks")
    mod._hook = None

    def set_axon_ntff_profile_hook(h):
        mod._hook = h

    def get_axon_ntff_profile_hook():
        return mod._hook

    mod.set_axon_ntff_profile_hook = set_axon_ntff_profile_hook
    mod.get_axon_ntff_profile_hook = get_axon_ntff_profile_hook
    sys.modules["antenv.axon_hooks"] = mod
    import antenv

    antenv.axon_hooks = mod
    so = "/opt/axon/libaxon_pjrt.so"
    if os.path.exists(so):
        try:
            from trn_agent_boot.trn_boot import _ntff_profile_via_ctypes

            set_axon_ntff_profile_hook(_ntff_profile_via_ctypes(so))
        except Exception as exc:  # pragma: no cover
            print(f"ntff hook registration failed: {exc}")
    import concourse.bass_utils as bu

    bu.upload_artifacts = lambda tmpdir: tmpdir


def kernel(x, mask, cos, sin, Wq, Wk, Wv, Wout):
    from concourse.bass_utils import run_bass_kernel_spmd

    nc = _get_nc()
    in_maps = shard_inputs(x, cos, sin, Wq, Wk, Wv, Wout)
    trace = bool(os.environ.get("BASS_KERNEL_TRACE"))
    if trace:
        _ensure_ntff_hook()
    res = run_bass_kernel_spmd(
        nc, in_maps, core_ids=list(range(N_CORES)), trace=trace,
        tmpdir=os.environ.get("BASS_KERNEL_TMPDIR"),
    )
    if trace and res.exec_time_ns is not None:
        print(f"HW exec time: {res.exec_time_ns} ns")
    out = np.zeros((S, OUT), dtype=np.float32)
    for d in range(N_CORES):
        out += res.results[d]["out_partial"].astype(np.float32)
    p = np.concatenate(
        [res.results[d]["p_out"].astype(np.float32) for d in range(N_CORES)], axis=0
    )
    return out[None], p[None]


# revision 16
# speedup vs baseline: 3.0342x; 1.4638x over previous
"""GQA attention (16 heads, 4 KV groups, S=2048, E=2048, HD=128) on 8 TRN2 cores.

Tensor-parallel by heads: core d owns heads {2d, 2d+1} and KV group d//2.
Each core computes its heads' attention + its slice of the output projection;
the host sums the 8 partial outputs (all-reduce) and concatenates the p slices.

Compute is bf16 on the TensorEngine (fp32 PSUM accumulation); RoPE and softmax
statistics are fp32. Outputs are stored bf16 and upconverted on host (verified
~0.5% rel err, well under the 2e-2 gate).

Outputs per core:
  p_out       [2, S, S] bf16 -- softmax probs for the core's 2 heads (causal;
                                the upper triangle is never written and relies
                                on the runtime pre-zeroing output buffers)
  out_partial [S, OUT]  bf16 -- this core's partial of o @ Wout
"""

import math
import os
from contextlib import ExitStack

import ml_dtypes
import numpy as np

import concourse.bass as bass
import concourse.bacc as bacc
import concourse.mybir as mybir
import concourse.tile as tile
from concourse.masks import make_causal_mask, make_identity

F32 = mybir.dt.float32
BF16 = mybir.dt.bfloat16
AF = mybir.ActivationFunctionType

# Problem constants (hardcoded per the grading contract).
S = 2048          # sequence length
E = 2048          # embedding dim
HD = 128          # head dim
H = 16            # total heads
G = 4             # kv groups
N_CORES = 8
H_LOC = H // N_CORES       # 2 heads per core
OUT = H * HD               # 2048
P = 128                    # partitions
SCALE = 1.0 / math.sqrt(HD)
NEG = -1e9                 # additive causal mask value (pre-scale)


def build_nc(s=S, e=E):
    """Build the single-core Bass program (identical across cores; per-core
    behaviour comes entirely from the data each core receives)."""
    nt = s // P
    et = e // P
    qkvw = H_LOC * HD + 2 * HD          # fused QKV projection width: 512
    nq = H_LOC * HD                      # 256

    nc = bacc.Bacc(None)
    xT = nc.declare_dram_parameter("xT", [e, s], BF16, isOutput=False)
    w_qkv = nc.declare_dram_parameter("w_qkv", [e, qkvw], BF16, isOutput=False)
    w_out = nc.declare_dram_parameter("w_out", [nq, s], BF16, isOutput=False)
    cos_d = nc.declare_dram_parameter("cos", [s, HD], F32, isOutput=False)
    sin_d = nc.declare_dram_parameter("sin", [s, HD], F32, isOutput=False)
    p_out = nc.declare_dram_parameter("p_out", [H_LOC, s, s], BF16, isOutput=True)
    out_partial = nc.declare_dram_parameter("out_partial", [s, s], BF16, isOutput=True)

    xT_t = xT[:].rearrange("(a p) m -> p a m", p=P)          # [128, et, s]
    wqkv_t = w_qkv[:].rearrange("(a p) m -> p a m", p=P)     # [128, et, 512]
    wout_t = w_out[:].rearrange("(a p) m -> p a m", p=P)     # [128, 2, s]
    cos_t = cos_d[:].rearrange("(a p) m -> p a m", p=P)      # [128, nt, 128]
    sin_t = sin_d[:].rearrange("(a p) m -> p a m", p=P)

    with tile.TileContext(nc) as tc, ExitStack() as ctx:
        consts = ctx.enter_context(tc.tile_pool(name="consts", bufs=1))
        persist = ctx.enter_context(tc.tile_pool(name="persist", bufs=1))
        xin = ctx.enter_context(tc.tile_pool(name="xin", bufs=2))
        rope = ctx.enter_context(tc.tile_pool(name="rope", bufs=2))
        epool = ctx.enter_context(tc.tile_pool(name="epool", bufs=2))
        ppool = ctx.enter_context(tc.tile_pool(name="ppool", bufs=2))
        ptpool = ctx.enter_context(tc.tile_pool(name="ptpool", bufs=2))
        small = ctx.enter_context(tc.tile_pool(name="small", bufs=4))
        ostage = ctx.enter_context(tc.tile_pool(name="ostage", bufs=3))
        ps_mm = ctx.enter_context(tc.tile_pool(name="ps_mm", bufs=4, space="PSUM"))
        ps_o = ctx.enter_context(tc.tile_pool(name="ps_o", bufs=2, space="PSUM"))
        ps_t = ctx.enter_context(tc.tile_pool(name="ps_t", bufs=2, space="PSUM"))

        # ---- constants ----
        wqkv_sb = consts.tile([P, et, qkvw], BF16)
        nc.sync.dma_start(out=wqkv_sb, in_=wqkv_t)
        cos_sb = consts.tile([P, nt, HD], F32)
        nc.sync.dma_start(out=cos_sb, in_=cos_t)
        sin_sb = consts.tile([P, nt, HD], F32)
        nc.sync.dma_start(out=sin_sb, in_=sin_t)
        ident = consts.tile([P, P], BF16)
        make_identity(nc, ident)
        cmask = consts.tile([P, P], F32)
        make_causal_mask(nc, cmask, mask_val=NEG)

        # ---- persistent activations (bf16) ----
        qt_sb = persist.tile([P, H_LOC, s], BF16)   # Q'^T per head  [HD, s]
        kt_sb = persist.tile([P, s], BF16)          # K'^T           [HD, s]
        v_sb = persist.tile([P, nt, HD], BF16)      # V natural      [s, HD]
        ot_sb = persist.tile([P, H_LOC, s], BF16)   # o^T per head   [HD, s]

        # ================= Phase A: QKV projection + RoPE + transposes ======
        for i in range(nt):
            xcol = xin.tile([P, et, P], BF16)
            nc.sync.dma_start(out=xcol, in_=xT_t[:, :, i * P:(i + 1) * P])
            ps = ps_mm.tile([P, 512], F32, tag="ps_mm")
            for ei in range(et):
                nc.tensor.matmul(
                    ps[:, :qkvw], xcol[:, ei, :], wqkv_sb[:, ei, :],
                    start=(ei == 0), stop=(ei == et - 1),
                )
            # RoPE (fp32) on the Q (2 heads) + K segments; V is a plain copy.
            qk = rope.tile([P, nq + HD], F32)
            rot = rope.tile([P, nq + HD], F32)
            qkb = rope.tile([P, nq + HD], BF16)
            for j in range(H_LOC + 1):
                seg = ps[:, j * HD:(j + 1) * HD]
                c = cos_sb[:, i, :]
                sn = sin_sb[:, i, :]
                h2 = HD // 2
                # rot = [q_hi, q_lo] * sin_eff  (sin first half pre-negated on host)
                nc.vector.tensor_mul(rot[:, j * HD:j * HD + h2], seg[:, h2:], sn[:, :h2])
                nc.vector.tensor_mul(rot[:, j * HD + h2:(j + 1) * HD], seg[:, :h2], sn[:, h2:])
                nc.vector.tensor_mul(qk[:, j * HD:(j + 1) * HD], seg, c)
            nc.vector.tensor_add(qkb, qk, rot)
            nc.scalar.copy(v_sb[:, i, :], ps[:, nq + HD:])
            # Transpose rope'd Q/K blocks into [HD, seq] layouts (PE + DVE).
            pst = ps_t.tile([P, 4 * P], BF16, tag="pst")
            for j in range(H_LOC + 1):
                nc.tensor.transpose(
                    pst[:, j * P:(j + 1) * P], qkb[:, j * HD:(j + 1) * HD], ident
                )
            nc.vector.tensor_copy(
                qt_sb[:, :, i * P:(i + 1) * P],
                pst[:, 0:H_LOC * P].rearrange("p (a m) -> p a m", m=P),
            )
            nc.vector.tensor_copy(
                kt_sb[:, i * P:(i + 1) * P], pst[:, H_LOC * P:(H_LOC + 1) * P]
            )

        # ================= Phase B: attention per head ======================
        for h in range(H_LOC):
            for qi in range(nt):
                klen = (qi + 1) * P
                nch = (klen + 511) // 512
                esb = epool.tile([P, s], BF16, tag="esb")
                acc = small.tile([P, 4], F32)
                for c in range(nch):
                    n0 = c * 512
                    ncw = min(512, klen - n0)
                    ps = ps_mm.tile([P, 512], F32, tag="ps_mm")
                    nc.tensor.matmul(
                        ps[:, :ncw],
                        qt_sb[:, h, qi * P:(qi + 1) * P],
                        kt_sb[:, n0:n0 + ncw],
                        start=True, stop=True,
                    )
                    if c == nch - 1:
                        nc.vector.tensor_add(ps[:, ncw - P:ncw], ps[:, ncw - P:ncw], cmask)
                    nc.scalar.activation(
                        esb[:, n0:n0 + ncw], ps[:, :ncw], AF.Exp,
                        scale=SCALE, accum_out=acc[:, c:c + 1],
                    )
                den = small.tile([P, 1], F32)
                nc.vector.reduce_sum(den, acc[:, :nch], axis=mybir.AxisListType.X)
                rden = small.tile([P, 1], F32)
                nc.vector.reciprocal(rden, den)
                pbf = ppool.tile([P, s], BF16, tag="pbf")
                nc.vector.tensor_scalar_mul(pbf[:, :klen], esb[:, :klen], rden)
                nc.sync.dma_start(
                    out=p_out[h, qi * P:(qi + 1) * P, 0:klen], in_=pbf[:, :klen]
                )
                # Transpose P into the [k, q] staging buffer (PE, batched copies).
                if qi % 2 == 0:
                    pt = ptpool.tile([P, nt, 2 * P], BF16, tag="pt")
                    nc.gpsimd.memset(pt[:, qi + 1, 0:P], 0.0)
                half = (qi % 2) * P
                for kt0 in range(0, qi + 1, 4):
                    kn = min(4, qi + 1 - kt0)
                    pst = ps_t.tile([P, 4 * P], BF16, tag="pst")
                    for k in range(kn):
                        nc.tensor.transpose(
                            pst[:, k * P:(k + 1) * P],
                            pbf[:, (kt0 + k) * P:(kt0 + k + 1) * P], ident,
                        )
                    nc.vector.tensor_copy(
                        pt[:, kt0:kt0 + kn, half:half + P],
                        pst[:, 0:kn * P].rearrange("p (a m) -> p a m", m=P),
                    )
                if qi % 2 == 1:
                    m = qi // 2
                    pso = ps_o.tile([P, 2 * P], F32)
                    for kt in range(qi + 1):
                        nc.tensor.matmul(
                            pso, v_sb[:, kt, :], pt[:, kt, :],
                            start=(kt == 0), stop=(kt == qi),
                        )
                    nc.scalar.copy(ot_sb[:, h, m * 2 * P:(m + 1) * 2 * P], pso)

        # ================= Phase C: output projection =======================
        wout_sb = consts.tile([P, nq // P, s], BF16)
        nc.sync.dma_start(out=wout_sb, in_=wout_t)
        for qi in range(nt):
            for c in range(s // 512):
                ps = ps_mm.tile([P, 512], F32, tag="ps_mm")
                for t in range(nq // P):
                    nc.tensor.matmul(
                        ps, ot_sb[:, t, qi * P:(qi + 1) * P],
                        wout_sb[:, t, c * 512:(c + 1) * 512],
                        start=(t == 0), stop=(t == nq // P - 1),
                    )
                ost = ostage.tile([P, 512], BF16)
                if (qi * (s // 512) + c) % 2 == 0:
                    nc.scalar.copy(ost, ps)
                else:
                    nc.vector.tensor_copy(ost, ps)
                nc.sync.dma_start(
                    out=out_partial[qi * P:(qi + 1) * P, c * 512:(c + 1) * 512],
                    in_=ost,
                )

    return nc


_NC_CACHE = {}


def _get_nc():
    key = "full"
    if key not in _NC_CACHE:
        nc = build_nc()
        nc.finalize()
        _NC_CACHE[key] = nc
    return _NC_CACHE[key]


def shard_inputs(x, cos, sin, Wq, Wk, Wv, Wout):
    """Host-side sharding: per-core input dicts (tensor-parallel by heads)."""
    bf = ml_dtypes.bfloat16
    xT = np.ascontiguousarray(np.asarray(x, dtype=np.float32)[0].T.astype(bf))
    cos = np.ascontiguousarray(np.asarray(cos, dtype=np.float32))
    sin = np.asarray(sin, dtype=np.float32)
    h2 = HD // 2
    sin_eff = np.ascontiguousarray(
        np.concatenate([-sin[:, :h2], sin[:, h2:]], axis=1)
    )
    Wq = np.asarray(Wq, dtype=np.float32)
    Wk = np.asarray(Wk, dtype=np.float32)
    Wv = np.asarray(Wv, dtype=np.float32)
    Wout = np.asarray(Wout, dtype=np.float32)
    in_maps = []
    for d in range(N_CORES):
        h0 = d * H_LOC
        g = h0 * G // H
        w_qkv = np.ascontiguousarray(np.concatenate(
            [
                Wq[:, h0 * HD:(h0 + H_LOC) * HD],
                Wk[:, g * HD:(g + 1) * HD],
                Wv[:, g * HD:(g + 1) * HD],
            ],
            axis=1,
        ).astype(bf))
        w_out = np.ascontiguousarray(Wout[h0 * HD:(h0 + H_LOC) * HD, :].astype(bf))
        in_maps.append({
            "xT": xT, "w_qkv": w_qkv, "w_out": w_out,
            "cos": cos, "sin": sin_eff,
        })
    return in_maps


def _ensure_ntff_hook():
    """Register the axon NTFF profile hook if the image's antenv lacks it."""
    import sys
    import types

    try:
        from antenv.axon_hooks import get_axon_ntff_profile_hook  # noqa: F401
        return
    except ImportError:
        pass
    mod = types.ModuleType("antenv.axon_hooks")
    mod._hook = None

    def set_axon_ntff_profile_hook(h):
        mod._hook = h

    def get_axon_ntff_profile_hook():
        return mod._hook

    mod.set_axon_ntff_profile_hook = set_axon_ntff_profile_hook
    mod.get_axon_ntff_profile_hook = get_axon_ntff_profile_hook
    sys.modules["antenv.axon_hooks"] = mod
    import antenv

    antenv.axon_hooks = mod
    so = "/opt/axon/libaxon_pjrt.so"
    if os.path.exists(so):
        try:
            from trn_agent_boot.trn_boot import _ntff_profile_via_ctypes

            set_axon_ntff_profile_hook(_ntff_profile_via_ctypes(so))
        except Exception as exc:  # pragma: no cover
            print(f"ntff hook registration failed: {exc}")
    import concourse.bass_utils as bu

    bu.upload_artifacts = lambda tmpdir: tmpdir


def kernel(x, mask, cos, sin, Wq, Wk, Wv, Wout):
    from concourse.bass_utils import run_bass_kernel_spmd

    nc = _get_nc()
    in_maps = shard_inputs(x, cos, sin, Wq, Wk, Wv, Wout)
    trace = bool(os.environ.get("BASS_KERNEL_TRACE"))
    if trace:
        _ensure_ntff_hook()
    res = run_bass_kernel_spmd(
        nc, in_maps, core_ids=list(range(N_CORES)), trace=trace,
        tmpdir=os.environ.get("BASS_KERNEL_TMPDIR"),
    )
    if trace and res.exec_time_ns is not None:
        print(f"HW exec time: {res.exec_time_ns} ns")
    out = np.zeros((S, OUT), dtype=np.float32)
    for d in range(N_CORES):
        out += res.results[d]["out_partial"].astype(np.float32)
    p = np.concatenate(
        [res.results[d]["p_out"].astype(np.float32) for d in range(N_CORES)], axis=0
    )
    return out[None], p[None]


# revision 17
# speedup vs baseline: 3.2951x; 1.0860x over previous
"""GQA attention (16 heads, 4 KV groups, S=2048, E=2048, HD=128) on 8 TRN2 cores.

Tensor-parallel by heads: core d owns heads {2d, 2d+1} and KV group d//2.
Each core computes its heads' attention + its slice of the output projection;
the host sums the 8 partial outputs (all-reduce) and concatenates the p slices.

Compute is bf16 on the TensorEngine (fp32 PSUM accumulation); RoPE and softmax
statistics are fp32. Outputs are stored bf16 and upconverted on host (verified
~0.5% rel err, well under the 2e-2 gate).

Outputs per core:
  p_out       [2, S, S] bf16 -- softmax probs for the core's 2 heads (causal;
                                the upper triangle is never written and relies
                                on the runtime pre-zeroing output buffers)
  out_partial [S, OUT]  bf16 -- this core's partial of o @ Wout
"""

import math
import os
from contextlib import ExitStack

import ml_dtypes
import numpy as np

import concourse.bass as bass
import concourse.bacc as bacc
import concourse.mybir as mybir
import concourse.tile as tile
from concourse.masks import make_causal_mask, make_identity

F32 = mybir.dt.float32
BF16 = mybir.dt.bfloat16
AF = mybir.ActivationFunctionType

# Problem constants (hardcoded per the grading contract).
S = 2048          # sequence length
E = 2048          # embedding dim
HD = 128          # head dim
H = 16            # total heads
G = 4             # kv groups
N_CORES = 8
H_LOC = H // N_CORES       # 2 heads per core
OUT = H * HD               # 2048
P = 128                    # partitions
SCALE = 1.0 / math.sqrt(HD)
NEG = -1e9                 # additive causal mask value (pre-scale)


def build_nc(s=S, e=E):
    """Build the single-core Bass program (identical across cores; per-core
    behaviour comes entirely from the data each core receives)."""
    nt = s // P
    et = e // P
    qkvw = H_LOC * HD + 2 * HD          # fused QKV projection width: 512
    nq = H_LOC * HD                      # 256

    nc = bacc.Bacc(None)
    xT = nc.declare_dram_parameter("xT", [e, s], BF16, isOutput=False)
    w_qkv = nc.declare_dram_parameter("w_qkv", [e, qkvw], BF16, isOutput=False)
    w_out = nc.declare_dram_parameter("w_out", [nq, s], BF16, isOutput=False)
    cos_d = nc.declare_dram_parameter("cos", [s, HD], F32, isOutput=False)
    sin_d = nc.declare_dram_parameter("sin", [s, HD], F32, isOutput=False)
    p_out = nc.declare_dram_parameter("p_out", [H_LOC, s, s], BF16, isOutput=True)
    out_partial = nc.declare_dram_parameter("out_partial", [s, s], BF16, isOutput=True)

    xT_t = xT[:].rearrange("(a p) m -> p a m", p=P)          # [128, et, s]
    wqkv_t = w_qkv[:].rearrange("(a p) m -> p a m", p=P)     # [128, et, 512]
    wout_t = w_out[:].rearrange("(a p) m -> p a m", p=P)     # [128, 2, s]
    cos_t = cos_d[:].rearrange("(a p) m -> p a m", p=P)      # [128, nt, 128]
    sin_t = sin_d[:].rearrange("(a p) m -> p a m", p=P)

    with tile.TileContext(nc) as tc, ExitStack() as ctx:
        consts = ctx.enter_context(tc.tile_pool(name="consts", bufs=1))
        persist = ctx.enter_context(tc.tile_pool(name="persist", bufs=1))
        xin = ctx.enter_context(tc.tile_pool(name="xin", bufs=3))
        rope = ctx.enter_context(tc.tile_pool(name="rope", bufs=2))
        epool = ctx.enter_context(tc.tile_pool(name="epool", bufs=3))
        ppool = ctx.enter_context(tc.tile_pool(name="ppool", bufs=3))
        ptpool = ctx.enter_context(tc.tile_pool(name="ptpool", bufs=2))
        small = ctx.enter_context(tc.tile_pool(name="small", bufs=4))
        ostage = ctx.enter_context(tc.tile_pool(name="ostage", bufs=3))
        ps_mm = ctx.enter_context(tc.tile_pool(name="ps_mm", bufs=4, space="PSUM"))
        ps_o = ctx.enter_context(tc.tile_pool(name="ps_o", bufs=2, space="PSUM"))
        ps_t = ctx.enter_context(tc.tile_pool(name="ps_t", bufs=2, space="PSUM"))

        # ---- constants ----
        wqkv_sb = consts.tile([P, et, qkvw], BF16)
        for wc in range(4):
            lo, hi = wc * et // 4, (wc + 1) * et // 4
            nc.sync.dma_start(out=wqkv_sb[:, lo:hi, :], in_=wqkv_t[:, lo:hi, :])
        cos_sb = consts.tile([P, nt, HD], F32)
        sin_sb = consts.tile([P, nt, HD], F32)
        for wc in range(2):
            lo, hi = wc * nt // 2, (wc + 1) * nt // 2
            nc.sync.dma_start(out=cos_sb[:, lo:hi, :], in_=cos_t[:, lo:hi, :])
            nc.sync.dma_start(out=sin_sb[:, lo:hi, :], in_=sin_t[:, lo:hi, :])
        ident = consts.tile([P, P], BF16)
        make_identity(nc, ident)
        cmask = consts.tile([P, P], F32)
        make_causal_mask(nc, cmask, mask_val=NEG)

        # ---- persistent activations (bf16) ----
        qt_sb = persist.tile([P, H_LOC, s], BF16)   # Q'^T per head  [HD, s]
        kt_sb = persist.tile([P, s], BF16)          # K'^T           [HD, s]
        v_sb = persist.tile([P, nt, HD], BF16)      # V natural      [s, HD]
        ot_sb = persist.tile([P, H_LOC, s], BF16)   # o^T per head   [HD, s]

        # ================= Phase A: QKV projection + RoPE + transposes ======
        for i in range(nt):
            xcol = xin.tile([P, et, P], BF16)
            nc.sync.dma_start(out=xcol, in_=xT_t[:, :, i * P:(i + 1) * P])
            ps = ps_mm.tile([P, 512], F32, tag="ps_mm")
            for ei in range(et):
                nc.tensor.matmul(
                    ps[:, :qkvw], xcol[:, ei, :], wqkv_sb[:, ei, :],
                    start=(ei == 0), stop=(ei == et - 1),
                )
            # RoPE (fp32) on the Q (2 heads) + K segments; V is a plain copy.
            qk = rope.tile([P, nq + HD], F32)
            rot = rope.tile([P, nq + HD], F32)
            qkb = rope.tile([P, nq + HD], BF16)
            for j in range(H_LOC + 1):
                seg = ps[:, j * HD:(j + 1) * HD]
                c = cos_sb[:, i, :]
                sn = sin_sb[:, i, :]
                h2 = HD // 2
                # rot = [q_hi, q_lo] * sin_eff  (sin first half pre-negated on host)
                nc.vector.tensor_mul(rot[:, j * HD:j * HD + h2], seg[:, h2:], sn[:, :h2])
                nc.vector.tensor_mul(rot[:, j * HD + h2:(j + 1) * HD], seg[:, :h2], sn[:, h2:])
                nc.vector.tensor_mul(qk[:, j * HD:(j + 1) * HD], seg, c)
            nc.vector.tensor_add(qkb, qk, rot)
            nc.scalar.copy(v_sb[:, i, :], ps[:, nq + HD:])
            # Transpose rope'd Q/K blocks into [HD, seq] layouts (PE + DVE).
            pst = ps_t.tile([P, 4 * P], BF16, tag="pst")
            for j in range(H_LOC + 1):
                nc.tensor.transpose(
                    pst[:, j * P:(j + 1) * P], qkb[:, j * HD:(j + 1) * HD], ident
                )
            nc.vector.tensor_copy(
                qt_sb[:, :, i * P:(i + 1) * P],
                pst[:, 0:H_LOC * P].rearrange("p (a m) -> p a m", m=P),
            )
            nc.vector.tensor_copy(
                kt_sb[:, i * P:(i + 1) * P], pst[:, H_LOC * P:(H_LOC + 1) * P]
            )

        # ================= Phase B: attention per head ======================
        for h in range(H_LOC):
            for qi in range(nt):
                klen = (qi + 1) * P
                nch = (klen + 511) // 512
                esb = epool.tile([P, s], BF16, tag="esb")
                acc = small.tile([P, 4], F32)
                for c in range(nch):
                    n0 = c * 512
                    ncw = min(512, klen - n0)
                    ps = ps_mm.tile([P, 512], F32, tag="ps_mm")
                    nc.tensor.matmul(
                        ps[:, :ncw],
                        qt_sb[:, h, qi * P:(qi + 1) * P],
                        kt_sb[:, n0:n0 + ncw],
                        start=True, stop=True,
                    )
                    if c == nch - 1:
                        nc.vector.tensor_add(ps[:, ncw - P:ncw], ps[:, ncw - P:ncw], cmask)
                    nc.scalar.activation(
                        esb[:, n0:n0 + ncw], ps[:, :ncw], AF.Exp,
                        scale=SCALE, accum_out=acc[:, c:c + 1],
                    )
                den = small.tile([P, 1], F32)
                nc.vector.reduce_sum(den, acc[:, :nch], axis=mybir.AxisListType.X)
                rden = small.tile([P, 1], F32)
                nc.vector.reciprocal(rden, den)
                pbf = ppool.tile([P, s], BF16, tag="pbf")
                nc.vector.tensor_scalar_mul(pbf[:, :klen], esb[:, :klen], rden)
                nc.sync.dma_start(
                    out=p_out[h, qi * P:(qi + 1) * P, 0:klen], in_=pbf[:, :klen]
                )
                # Transpose P into the [k, q] staging buffer (PE, batched copies).
                if qi % 2 == 0:
                    pt = ptpool.tile([P, nt, 2 * P], BF16, tag="pt")
                    nc.gpsimd.memset(pt[:, qi + 1, 0:P], 0.0)
                half = (qi % 2) * P
                for kt0 in range(0, qi + 1, 4):
                    kn = min(4, qi + 1 - kt0)
                    pst = ps_t.tile([P, 4 * P], BF16, tag="pst")
                    for k in range(kn):
                        nc.tensor.transpose(
                            pst[:, k * P:(k + 1) * P],
                            pbf[:, (kt0 + k) * P:(kt0 + k + 1) * P], ident,
                        )
                    nc.vector.tensor_copy(
                        pt[:, kt0:kt0 + kn, half:half + P],
                        pst[:, 0:kn * P].rearrange("p (a m) -> p a m", m=P),
                    )
                if qi % 2 == 1:
                    m = qi // 2
                    pso = ps_o.tile([P, 2 * P], F32)
                    for kt in range(qi + 1):
                        nc.tensor.matmul(
                            pso, v_sb[:, kt, :], pt[:, kt, :],
                            start=(kt == 0), stop=(kt == qi),
                        )
                    nc.scalar.copy(ot_sb[:, h, m * 2 * P:(m + 1) * 2 * P], pso)

        # ================= Phase C: output projection =======================
        wout_sb = consts.tile([P, nq // P, s], BF16)
        nc.sync.dma_start(out=wout_sb, in_=wout_t)
        for qi in range(nt):
            for c in range(s // 512):
                ps = ps_mm.tile([P, 512], F32, tag="ps_mm")
                for t in range(nq // P):
                    nc.tensor.matmul(
                        ps, ot_sb[:, t, qi * P:(qi + 1) * P],
                        wout_sb[:, t, c * 512:(c + 1) * 512],
                        start=(t == 0), stop=(t == nq // P - 1),
                    )
                ost = ostage.tile([P, 512], BF16)
                if (qi * (s // 512) + c) % 2 == 0:
                    nc.scalar.copy(ost, ps)
                else:
                    nc.vector.tensor_copy(ost, ps)
                nc.sync.dma_start(
                    out=out_partial[qi * P:(qi + 1) * P, c * 512:(c + 1) * 512],
                    in_=ost,
                )

    return nc


_NC_CACHE = {}


def _get_nc():
    key = "full"
    if key not in _NC_CACHE:
        nc = build_nc()
        nc.finalize()
        _NC_CACHE[key] = nc
    return _NC_CACHE[key]


def shard_inputs(x, cos, sin, Wq, Wk, Wv, Wout):
    """Host-side sharding: per-core input dicts (tensor-parallel by heads)."""
    bf = ml_dtypes.bfloat16
    xT = np.ascontiguousarray(np.asarray(x, dtype=np.float32)[0].T.astype(bf))
    cos = np.ascontiguousarray(np.asarray(cos, dtype=np.float32))
    sin = np.asarray(sin, dtype=np.float32)
    h2 = HD // 2
    sin_eff = np.ascontiguousarray(
        np.concatenate([-sin[:, :h2], sin[:, h2:]], axis=1)
    )
    Wq = np.asarray(Wq, dtype=np.float32)
    Wk = np.asarray(Wk, dtype=np.float32)
    Wv = np.asarray(Wv, dtype=np.float32)
    Wout = np.asarray(Wout, dtype=np.float32)
    in_maps = []
    for d in range(N_CORES):
        h0 = d * H_LOC
        g = h0 * G // H
        w_qkv = np.ascontiguousarray(np.concatenate(
            [
                Wq[:, h0 * HD:(h0 + H_LOC) * HD],
                Wk[:, g * HD:(g + 1) * HD],
                Wv[:, g * HD:(g + 1) * HD],
            ],
            axis=1,
        ).astype(bf))
        w_out = np.ascontiguousarray(Wout[h0 * HD:(h0 + H_LOC) * HD, :].astype(bf))
        in_maps.append({
            "xT": xT, "w_qkv": w_qkv, "w_out": w_out,
            "cos": cos, "sin": sin_eff,
        })
    return in_maps


def _ensure_ntff_hook():
    """Register the axon NTFF profile hook if the image's antenv lacks it."""
    import sys
    import types

    try:
        from antenv.axon_hooks import get_axon_ntff_profile_hook  # noqa: F401
        return
    except ImportError:
        pass
    mod = types.ModuleType("antenv.axon_hooks")
    mod._hook = None

    def set_axon_ntff_profile_hook(h):
        mod._hook = h

    def get_axon_ntff_profile_hook():
        return mod._hook

    mod.set_axon_ntff_profile_hook = set_axon_ntff_profile_hook
    mod.get_axon_ntff_profile_hook = get_axon_ntff_profile_hook
    sys.modules["antenv.axon_hooks"] = mod
    import antenv

    antenv.axon_hooks = mod
    so = "/opt/axon/libaxon_pjrt.so"
    if os.path.exists(so):
        try:
            from trn_agent_boot.trn_boot import _ntff_profile_via_ctypes

            set_axon_ntff_profile_hook(_ntff_profile_via_ctypes(so))
        except Exception as exc:  # pragma: no cover
            print(f"ntff hook registration failed: {exc}")
    import concourse.bass_utils as bu

    bu.upload_artifacts = lambda tmpdir: tmpdir


def kernel(x, mask, cos, sin, Wq, Wk, Wv, Wout):
    from concourse.bass_utils import run_bass_kernel_spmd

    nc = _get_nc()
    in_maps = shard_inputs(x, cos, sin, Wq, Wk, Wv, Wout)
    trace = bool(os.environ.get("BASS_KERNEL_TRACE"))
    if trace:
        _ensure_ntff_hook()
    res = run_bass_kernel_spmd(
        nc, in_maps, core_ids=list(range(N_CORES)), trace=trace,
        tmpdir=os.environ.get("BASS_KERNEL_TMPDIR"),
    )
    if trace and res.exec_time_ns is not None:
        print(f"HW exec time: {res.exec_time_ns} ns")
    out = np.zeros((S, OUT), dtype=np.float32)
    for d in range(N_CORES):
        out += res.results[d]["out_partial"].astype(np.float32)
    p = np.concatenate(
        [res.results[d]["p_out"].astype(np.float32) for d in range(N_CORES)], axis=0
    )
    return out[None], p[None]
